# revision 32
# baseline (speedup 1.0000x reference)
"""Trainium2 Bass kernel for nn_LinearLayer_45243185496808.

Computes out[b,o] = sum_i tanh(x[b,i]*t) * w[o,i], w = sum_p coef[o,i,p],
with B=131072, I=O=128, data-parallel over batch on 8 NeuronCores.

v3 pipeline (B_CORE=16384 rows/core), built from NTFF trace analysis:
  - w is reduced over p and pre-transposed on the host (a parameter
    reshape; 32 KiB f16 instead of 1 MiB f32 of per-core HBM traffic).
  - The HBM window dominates (~12.6 MiB/core; SWDGE sustains ~425 GB/s
    read-side when loads run alone, ~400 GB/s mixed). Fixed costs:
    ~7 us engine-start preamble, ~2.5 us epilogue.
  - ALL x loads ride the gpsimd SWDGE queue as f32->f16 casting DMAs
    (halves SBUF-fabric bytes; HWDGE cannot cast). The sync HWDGE queue
    carries only the tiny consts (w_T, identity) EARLY - anything queued
    on HWDGE after the SWDGE stream starts is starved to a trickle by
    the SDMA packet round-robin (measured: a 64 KiB const took 12+ us,
    stalling all compute, since every transpose reads the identity).
  - Graded chunk sizes: small head chunks so compute starts ~2 us
    earlier; small tail chunks so the last compute->store chain is short.
  - Per 128-row slice: PE transpose (f16, LDW transpose_mode) -> PSUM,
    ScalarE tanh -> SBUF f16, f16 matmul (N=128, stationary=values,
    moving=w_T) -> PSUM f32, DVE copy -> f16 out tile. The PE stream is
    software-pipelined (transposes of group g+1 emitted before matmuls
    of group g) because PE dispatch is strict FIFO and would otherwise
    idle-wait for each group's tanh.
  - All x/out SBUF tiles are resident simultaneously (no pool-reuse
    stalls); identity comes in as a host input so gpsimd's stream is
    pure load dispatches.
  - Stores ride sync HWDGE per computed piece; they trickle during the
    load phase (round-robin starved) and flush at full rate once loads
    finish - measured better than queuing them behind loads on the
    SWDGE ring, which makes the tail compute-dispatch-paced.
Accuracy vs f64 reference ~5e-4 absmax-relative (f16 in/out + f16 mults,
f32 PSUM accumulate).
"""

import os
import sys
import types

import numpy as np

import concourse.bass as bass
import concourse.mybir as mybir
import concourse.tile as tile
from concourse import bacc
from concourse.bass_utils import run_bass_kernel_spmd


def _ensure_ntff_hook():
    """Register the axon NTFF profile hook if the image lacks antenv.axon_hooks.

    Only needed for BASS_TRACE=1 profiling runs; harmless otherwise."""
    if "antenv.axon_hooks" in sys.modules:
        return
    try:
        from antenv.axon_hooks import get_axon_ntff_profile_hook  # noqa: F401

        return  # real module importable
    except ImportError:
        pass
    hook = None
    try:
        from trn_agent_boot.trn_boot import _ntff_profile_via_ctypes

        so_path = "/opt/axon/libaxon_pjrt.so"
        if os.path.exists(so_path):
            hook = _ntff_profile_via_ctypes(so_path)
    except Exception:
        hook = None
    mod = types.ModuleType("antenv.axon_hooks")
    mod.get_axon_ntff_profile_hook = lambda: hook
    mod.set_axon_ntff_profile_hook = lambda h: None
    sys.modules["antenv.axon_hooks"] = mod


N_CORES = 8
B_FULL = 131072
I_DIM = 128
O_DIM = 128
P_NUM = 16
P = 128                     # SBUF partitions
B_CORE = B_FULL // N_CORES  # 16384
G = 8                       # 128-row slices per act/PSUM group

_ROWS = [1024, 1024, 4096, 4096, 4096, 1024, 1024]
CHUNK_PLAN = []
_r0 = 0
for _r in _ROWS:
    CHUNK_PLAN.append((_r0, _r))
    _r0 += _r
assert _r0 == B_CORE

LAST_RESULT = None  # BassKernelResults of the most recent run (for test.py)


def build_bass(tanh_scale: float) -> bass.Bass:
    nc = bacc.Bacc("TRN2", target_bir_lowering=False)
    x = nc.dram_tensor("x", [B_CORE, I_DIM], mybir.dt.float32, kind="ExternalInput")
    wt = nc.dram_tensor("wt", [I_DIM, O_DIM], mybir.dt.float16, kind="ExternalInput")
    id16 = nc.dram_tensor("id16", [P, P], mybir.dt.float16, kind="ExternalInput")
    # Output leaves the device as f16 (halves store traffic; |out| << f16
    # range). Host upcasts back to f32.
    out = nc.dram_tensor("out", [B_CORE, O_DIM], mybir.dt.float16, kind="ExternalOutput")

    def chunk_view(t, row0, rows):
        return t[row0 : row0 + rows, :].rearrange("(p r) d -> p (r d)", p=P)

    with tile.TileContext(nc) as tc:
        with (
            tc.tile_pool(name="consts", bufs=1) as consts,
            tc.tile_pool(name="xin", bufs=1) as xin_pool,
            tc.tile_pool(name="vals", bufs=6) as vals_pool,
            tc.tile_pool(name="outp", bufs=1) as out_pool,
            tc.tile_pool(name="pxT", bufs=2, space="PSUM") as pxT_pool,
            tc.tile_pool(name="pout", bufs=3, space="PSUM") as pout_pool,
        ):
            # consts first on the sync HWDGE queue - they land before the
            # SWDGE load stream starts hogging the SDMA engines.
            wt_sb = consts.tile([P, O_DIM], mybir.dt.float16)
            nc.sync.dma_start(out=wt_sb[:], in_=wt[:, :])
            identity_h = consts.tile([P, P], mybir.dt.float16)
            nc.sync.dma_start(out=identity_h[:], in_=id16[:, :])

            x_tiles = []
            for c, (row0, rows) in enumerate(CHUNK_PLAN):
                x_sb = xin_pool.tile(
                    [P, (rows // P) * I_DIM], mybir.dt.float16, tag=f"x{c}"
                )
                x_tiles.append(x_sb)
                nc.gpsimd.dma_start(out=x_sb[:], in_=chunk_view(x, row0, rows))

            # --- main loop, software-pipelined on PE ---
            # PE dispatch is strict FIFO: emitting T(g),M(g),T(g+1),... makes
            # PE idle-wait for ACT(g) before M(g). Emit T(g+1) before M(g) so
            # PE transposes the next group while ScalarE runs tanh on this one.
            groups = []  # (c, slice0, out_sb)
            chunk_left = {}
            for c, (row0, rows) in enumerate(CHUNK_PLAN):
                n_slices = rows // P
                assert n_slices % G == 0
                out_sb = out_pool.tile(
                    [P, n_slices * O_DIM], mybir.dt.float16, tag=f"o{c}"
                )
                chunk_left[c] = n_slices
                for s0 in range(0, n_slices, G):
                    groups.append((c, s0, out_sb))

            n_groups = len(groups)
            stage = [None] * n_groups

            def emit_front(gi):
                c, s0, out_sb = groups[gi]
                x_sb = x_tiles[c]
                xT_ps = pxT_pool.tile([P, G * P], mybir.dt.float16, tag="xT_ps")
                for j in range(G):
                    n = s0 + j
                    nc.tensor.transpose(
                        xT_ps[:, j * P : (j + 1) * P],
                        x_sb[:, n * I_DIM : (n + 1) * I_DIM],
                        identity_h[:],
                    )
                v_T = vals_pool.tile([P, G * P], mybir.dt.float16, tag="v_T")
                nc.scalar.activation(
                    v_T[:],
                    xT_ps[:],
                    mybir.ActivationFunctionType.Tanh,
                    scale=tanh_scale,
                )
                stage[gi] = v_T

            def emit_back(gi):
                c, s0, out_sb = groups[gi]
                v_T = stage[gi]
                o_ps = pout_pool.tile([P, G * O_DIM], mybir.dt.float32, tag="o_ps")
                for j in range(G):
                    nc.tensor.matmul(
                        o_ps[:, j * O_DIM : (j + 1) * O_DIM],
                        v_T[:, j * P : (j + 1) * P],
                        wt_sb[:],
                        start=True,
                        stop=True,
                    )
                nc.vector.tensor_copy(
                    out_sb[:, s0 * O_DIM : (s0 + G) * O_DIM], o_ps[:]
                )
                chunk_left[c] -= G
                if chunk_left[c] == 0:  # chunk fully computed -> store it
                    # 512-row sub-pieces: store descriptors have 1 KiB
                    # per-partition runs vs the loads' 8 KiB, so the SDMA
                    # per-packet round-robin gives loads ~8/9 of the HBM bus
                    # instead of ~2/3 (loads pace the compute tail).
                    row0, rows = CHUNK_PLAN[c]
                    ov = chunk_view(out, row0, rows)
                    if c <= 3:
                        sub = 8 * O_DIM  # columns per 1024-row sub-piece
                        for q in range(rows // 1024):
                            nc.sync.dma_start(
                                out=ov[:, q * sub : (q + 1) * sub],
                                in_=out_sb[:, q * sub : (q + 1) * sub],
                            )
                    else:
                        # late chunks finish near/after load-end: dispatch
                        # from the idle gpsimd engine onto the SWDGE ring,
                        # where they queue directly behind the loads (the
                        # scalar engine's dispatches are FIFO-stuck behind
                        # the tanh stream until the very end)
                        nc.gpsimd.dma_start(out=ov[:], in_=out_sb[:])

            for gi in range(n_groups):
                emit_front(gi)
                if gi >= 1:
                    emit_back(gi - 1)
            emit_back(n_groups - 1)
    nc.finalize()
    return nc


def kernel(x, coef, tanh_range):
    global LAST_RESULT
    x = np.ascontiguousarray(np.asarray(x, dtype=np.float32))
    coef = np.asarray(coef, dtype=np.float32)
    t = float(np.asarray(tanh_range))
    assert x.shape == (B_FULL, I_DIM), x.shape
    assert coef.shape == (O_DIM, I_DIM, P_NUM), coef.shape

    # Parameter prep on host: w[o,i] = sum_p coef[o,i,p], laid out [i,o].
    wt = np.ascontiguousarray(coef.sum(axis=2).T.astype(np.float16))
    id16 = np.eye(P, dtype=np.float16)

    nc = build_bass(t)
    in_maps = [
        {
            "x": np.ascontiguousarray(x[k * B_CORE : (k + 1) * B_CORE]),
            "wt": wt,
            "id16": id16,
        }
        for k in range(N_CORES)
    ]
    if os.environ.get("BASS_TRACE"):
        _ensure_ntff_hook()
    res = run_bass_kernel_spmd(nc, in_maps, core_ids=list(range(N_CORES)))
    LAST_RESULT = res
    return np.concatenate(
        [r["out"].astype(np.float32) for r in res.results], axis=0
    )
